# revision 28
# baseline (speedup 1.0000x reference)
"""KAN expert (2x KANLinear, grid=5, order=3) on 8 TRN2 NeuronCores.

Algorithm
---------
KANLinear(x) = silu(x) @ base_w.T + einsum('big,oig->bo', B(x), spline_w*scaler)
where B(x) are 8 cubic B-spline bases on the uniform knot grid
t_j = -2.2 + 0.4*j, j=0..11.

Spline part reformulated with truncated powers: for x clamped to
(-inf, 2.2], B_g(x) = (1/(6 h^3)) * sum_{k=0..4} (-1)^k C(4,k) q_{g+k}(x)
with q_j(x) = relu(min(x, 2.2) - t_j)^3 and q_11 == 0. The 5-term
combination is folded into the spline weights on the host, so the device
computes, per feature, 12 channels (silu + q_0..q_10) and one matmul per
layer: out = concat_ch[silu(x), q_0..q_10] @ Wcat  (K = 12*in).

Sharding: data-parallel over batch (16384 -> 2048 rows/core), weights
replicated. Matmul operands fp16 (full PE rate), PSUM fp32.

Per-core pipeline over batch-column segments (256/512 wide), activations
transposed [feature, batch]; elementwise ops fused over feature blocks
([128, FB, seg] tiles). Per q channel: r = relu(xc-t) (DVE tensor_scalar,
4x mode), square s2 = (xc-t)^2 on ACT (Square with bias) or GPSIMD
(tt r*r), final q = s2*r on DVE or GPSIMD per a static balance table
that keeps every engine below the PE's ~31 us/chunk. Matmul emission is
software-pipelined: mm_l1(n+1) issues before mm_l2(n) so the PE never
waits for layer-2 channels of segment n (they are computed by
DVE/ACT/GPSIMD while mm_l1(n+1) streams).

This walrus build only supports ONE sync-wait per instruction; Tile
emits more. `_split_multiwait_json` rewrites the serialized BIR to hoist
excess waits onto injected NoOps.
"""

import json

import numpy as np

# ---------------------------------------------------------------- constants
B, IN, HID, OUT = 16384, 512, 256, 256
NCORES = 8
BPC = B // NCORES            # 2048 batch rows per core
# batch-column segments per core: narrow at the ends so the PE pipeline
# ramps in/out quickly, wide in the middle for low per-op overhead.
SEGS = [256, 512, 512, 512, 256]
assert sum(SEGS) == BPC
GRID_H = 0.4
T0 = -2.2
CLAMP_HI = 2.2
NCH = 11                     # truncated-power channels j = 0..10
KT1 = (1 + NCH) * IN // 128  # 48 k-tiles, layer 1
KT2 = (1 + NCH) * HID // 128  # 24 k-tiles, layer 2
FB1 = IN // 128              # 4 feature blocks, layer 1
FB2 = HID // 128             # 2 feature blocks, layer 2

_KNOTS = [T0 + GRID_H * j for j in range(12)]

# Per-channel engine assignment: (square_engine, mult_engine) per layer.
# square: 'A' = ACT Square(xc + bias); 'P' = GPSIMD TT(r, r).
# mult:   'D' = DVE tensor_tensor;     'P' = GPSIMD tensor_tensor.
# r = relu(xc - t_j) is always a DVE tensor_scalar.
RECIPES = {
    1: [("P", "D"), ("A", "P"), ("P", "D"), ("A", "D"), ("P", "D"),
        ("A", "P"), ("P", "D"), ("A", "D"), ("P", "D"), ("A", "P"),
        ("A", "D")],
    2: [("P", "D"), ("A", "D"), ("P", "D"), ("A", "P"), ("P", "D"),
        ("A", "D"), ("P", "D"), ("A", "P"), ("A", "D"), ("P", "D"),
        ("A", "D")],
}

_CACHE = {}


# ---------------------------------------------------------------- host math
def _fold_weights(spline_w, scaler):
    """[o,i,8] spline weights * scaler -> [o,i,11] truncated-power weights."""
    sw = spline_w.astype(np.float64) * scaler.astype(np.float64)[..., None]
    comb = np.array([1.0, -4.0, 6.0, -4.0, 1.0], np.float64) / (6.0 * GRID_H**3)
    o, i, _ = sw.shape
    wf = np.zeros((o, i, 12), np.float64)
    for g in range(8):
        for k in range(5):
            wf[:, :, g + k] += comb[k] * sw[:, :, g]
    return wf[:, :, :NCH]  # q_11 == 0 after the clamp


def _wcat_swizzled(base_w, spline_w, scaler):
    """Concat [base; folded-spline] weights, channel-major rows, swizzled to
    the SBUF layout [128, KT*out] (per-partition contiguous DMA)."""
    o, i = base_w.shape
    rows = np.empty((1 + NCH, i, o), np.float64)
    rows[0] = base_w.astype(np.float64).T
    rows[1:] = _fold_weights(spline_w, scaler).transpose(2, 1, 0)
    wcat = rows.reshape((1 + NCH) * i, o)               # [K, o]
    kt = wcat.shape[0] // 128
    return np.ascontiguousarray(
        wcat.reshape(kt, 128, o).transpose(1, 0, 2).reshape(128, kt * o)
    ).astype(np.float16)


def _x_swizzled(x):
    """x [B, IN] fp32 -> per-core [128, FB1*BPC] fp16 transposed layout."""
    xt = np.ascontiguousarray(x.T.astype(np.float16))   # [IN, B]
    outs = []
    for c in range(NCORES):
        sh = xt[:, c * BPC:(c + 1) * BPC]               # [512, 2048]
        outs.append(np.ascontiguousarray(
            sh.reshape(FB1, 128, BPC).transpose(1, 0, 2).reshape(128, FB1 * BPC)))
    return outs


# ------------------------------------------------- BIR wait-limit workaround
def _split_multiwait_json(raw: bytes) -> bytes:
    """Walrus in this container encodes at most one sync-wait per ISA
    instruction. Hoist extra waits onto injected same-engine NoOps."""
    d = json.loads(raw)
    k = 0
    for fn in d["functions"]:
        for bb in fn["blocks"]:
            out = []
            for ins in bb["instructions"]:
                si = ins.get("sync_info")
                waits = (si or {}).get("on_wait") or []
                if len(waits) > 1:
                    for w in waits[:-1]:
                        k += 1
                        out.append({"debug": ins.get("debug", 0),
                                    "engine": ins["engine"], "ins": [],
                                    "outs": [], "name": f"antws-{k}",
                                    "opcode": "NoOp",
                                    "sync_info": {"on_update": [],
                                                  "on_wait": [w]}})
                    si["on_wait"] = [waits[-1]]
                out.append(ins)
            bb["instructions"] = out
    return json.dumps(d).encode()


# ---------------------------------------------------------------- program
def _build_program():
    if "nc" in _CACHE:
        return _CACHE["nc"]
    import concourse.bass as bass
    import concourse.mybir as mybir
    import concourse.tile as tile

    f16 = mybir.dt.float16
    f32 = mybir.dt.float32
    AF = mybir.ActivationFunctionType
    ALU = mybir.AluOpType

    nc = bass.Bass(target_bir_lowering=False)

    # const APs for activation float biases (walrus wants APs, not imms)
    def reg_const(v):
        v = float(v)
        if (f32, v) in nc.const_aps.aps:
            return
        t = nc.alloc_sbuf_tensor(f"constk_{len(nc.const_aps.aps)}", [128, 1], f32)
        nc.gpsimd.memset(t.ap(), v)
        nc.const_aps.aps[(f32, v)] = t.ap()

    for j in range(NCH):
        reg_const(-_KNOTS[j])
    nc.all_engine_barrier()

    x_d = nc.declare_dram_parameter("xT", [128, FB1 * BPC], f16, isOutput=False)
    w1_d = nc.declare_dram_parameter("w1", [128, KT1 * OUT], f16, isOutput=False)
    w2_d = nc.declare_dram_parameter("w2", [128, KT2 * OUT], f16, isOutput=False)
    y_d = nc.declare_dram_parameter("yT", [OUT, BPC], f32, isOutput=True)

    with tile.TileContext(nc) as tc:
        with (
            tc.tile_pool(name="wp", bufs=1) as wp,
            tc.tile_pool(name="xp", bufs=1) as xp,
            tc.tile_pool(name="ch1", bufs=18) as ch1p,   # [128,FB1,512] tiles
            tc.tile_pool(name="ch2", bufs=16) as ch2p,   # [128,FB2,512] tiles
            tc.tile_pool(name="aux", bufs=4) as auxp,
            tc.tile_pool(name="outp", bufs=1) as outp,
            tc.tile_pool(name="ps", bufs=2, space="PSUM") as psp,
        ):
            w1_sb = wp.tile([128, KT1, OUT], f16, tag="w1")
            w2_sb = wp.tile([128, KT2, OUT], f16, tag="w2")
            x_sb = xp.tile([128, FB1, BPC], f16, tag="x")

            # One sync DMA queue: x segment 0 first (unblocks silu/q of the
            # first segment), then w1 in k-ordered eighths interleaved with
            # the remaining x segments so early k-tile weights land first.
            x_src = x_d.rearrange("p (f b) -> p f b", b=BPC)
            seg_lo = [sum(SEGS[:i]) for i in range(len(SEGS))]

            def dma_x(n):
                lo, w = seg_lo[n], SEGS[n]
                if n == 0:
                    # fb halves so segment 0's silu can start after 1 KiB
                    for h in range(2):
                        nc.sync.dma_start(
                            x_sb[:, 2 * h:2 * h + 2, lo:lo + w],
                            x_src[:, 2 * h:2 * h + 2, lo:lo + w])
                else:
                    nc.sync.dma_start(x_sb[:, :, lo:lo + w],
                                      x_src[:, :, lo:lo + w])

            def dma_w1(s):
                kl, kh = s * (KT1 // 8), (s + 1) * (KT1 // 8)
                nc.sync.dma_start(w1_sb[:, kl:kh, :],
                                  w1_d[:, kl * OUT:kh * OUT].rearrange(
                                      "p (k m) -> p k m", m=OUT))

            dma_x(0)
            dma_w1(0)
            dma_w1(1)
            dma_x(1)
            dma_w1(2)
            dma_w1(3)
            dma_x(2)
            for s in range(4, 8):
                dma_w1(s)
            for n in range(3, len(SEGS)):
                dma_x(n)
            for s in range(2):
                kl, kh = s * (KT2 // 2), (s + 1) * (KT2 // 2)
                nc.sync.dma_start(w2_sb[:, kl:kh, :],
                                  w2_d[:, kl * OUT:kh * OUT].rearrange(
                                      "p (k m) -> p k m", m=OUT))

            def make_channels(layer, xb, chp, nfb, width, lbl, ramp=False):
                """xb: [128, nfb, width] fp16 source tile AP.
                Returns list of 12 [128, nfb, width] channel tiles.
                ramp: emit silu/clamp in fb halves so the first matmuls
                unblock as soon as the first half of x lands."""
                tiles = []
                s = chp.tile([128, nfb, width], f16, tag="ch",
                             name=f"silu_{lbl}")
                xc = auxp.tile([128, nfb, width], f16, tag=f"xc{layer}",
                               name=f"xc_{lbl}", bufs=2)
                if ramp and nfb % 2 == 0:
                    h = nfb // 2
                    for i in range(2):
                        nc.scalar.activation(s[:, i * h:(i + 1) * h, :],
                                             xb[:, i * h:(i + 1) * h, :],
                                             AF.Silu)
                        nc.vector.tensor_scalar_min(
                            xc[:, i * h:(i + 1) * h, :],
                            xb[:, i * h:(i + 1) * h, :], CLAMP_HI)
                else:
                    nc.scalar.activation(s[:], xb, AF.Silu)
                    nc.vector.tensor_scalar_min(xc[:], xb, CLAMP_HI)
                tiles.append(s)
                for j in range(NCH):
                    sq_eng, mul_eng = RECIPES[layer][j]
                    q = chp.tile([128, nfb, width], f16, tag="ch",
                                 name=f"q_{lbl}_{j}")
                    r = auxp.tile([128, nfb, width], f16, tag=f"r{layer}",
                                  name=f"r_{lbl}_{j}", bufs=2)
                    nc.vector.tensor_scalar(r[:], xc[:], float(_KNOTS[j]),
                                            0.0, ALU.subtract, ALU.max)
                    s2 = auxp.tile([128, nfb, width], f16, tag=f"s2{layer}",
                                   name=f"s2_{lbl}_{j}", bufs=2)
                    if sq_eng == "A":
                        nc.scalar.activation(s2[:], xc[:], AF.Square,
                                             bias=float(-_KNOTS[j]))
                    else:
                        nc.gpsimd.tensor_tensor(s2[:], r[:], r[:], ALU.mult)
                    meng = nc.vector if mul_eng == "D" else nc.gpsimd
                    meng.tensor_tensor(q[:], s2[:], r[:], ALU.mult)
                    tiles.append(q)
                return tiles

            def mm(ps, w_sb, tiles, kt_count, nfb):
                for kt in range(kt_count):
                    ch, f = kt // nfb, kt % nfb
                    rhs = tiles[ch][:, f, :]
                    for m in range(2):
                        nc.tensor.matmul(ps[m][:],
                                         w_sb[:, kt, m * 128:(m + 1) * 128],
                                         rhs, start=(kt == 0),
                                         stop=(kt == kt_count - 1))

            def emit_l2_tail(n, ps2_of, m_outer=False):
                """mm_l2 for segment n, then y evacuation + store.
                m_outer: complete M-block 0's accumulation first so its
                evacuation overlaps M-block 1 (used for the last segment)."""
                ps2 = ps2_of[n]
                lo, w = seg_lo[n], SEGS[n]

                def evac(m):
                    yt = outp.tile([128, w], f32, tag=f"yt_{m}",
                                   name=f"yt_{n}_{m}")
                    nc.scalar.copy(yt[:], ps2[m][:])
                    nc.sync.dma_start(y_d[m * 128:(m + 1) * 128, lo:lo + w],
                                      yt[:])

                if m_outer:
                    for m in range(2):
                        for kt in range(KT2):
                            ch, f = kt // FB2, kt % FB2
                            nc.tensor.matmul(
                                ps2[m][:], w2_sb[:, kt, m * 128:(m + 1) * 128],
                                t2_of[n][ch][:, f, :], start=(kt == 0),
                                stop=(kt == KT2 - 1))
                        evac(m)
                else:
                    mm(ps2, w2_sb, t2_of[n], KT2, FB2)
                    for m in range(2):
                        evac(m)

            t2_of = {}
            ps2_of = {}
            for n, w in enumerate(SEGS):
                lo = seg_lo[n]
                # ---- layer-1 channels + matmul for segment n
                t1 = make_channels(1, x_sb[:, :, lo:lo + w], ch1p,
                                   FB1, w, f"l1c{n}", ramp=(n == 0))
                ps1 = [psp.tile([128, w], f32, tag=f"ps1_{m}",
                                name=f"ps1_{n}_{m}") for m in range(2)]
                mm(ps1, w1_sb, t1, KT1, FB1)
                # ---- layer-2 matmul of the PREVIOUS segment (PE
                # pipelining): its channels were computed while mm_l1(n)
                # streamed.
                if n >= 1:
                    emit_l2_tail(n - 1, ps2_of)
                # ---- evacuate ps1 -> x2, then layer-2 channels for seg n
                x2 = auxp.tile([128, FB2, w], f16, tag="x2",
                               name=f"x2_{n}", bufs=2)
                for m in range(2):
                    nc.scalar.copy(x2[:, m, :], ps1[m][:])
                t2_of[n] = make_channels(2, x2[:], ch2p, FB2, w, f"l2c{n}")
                ps2_of[n] = [psp.tile([128, w], f32, tag=f"ps2_{m}",
                                      name=f"ps2_{n}_{m}") for m in range(2)]
            emit_l2_tail(len(SEGS) - 1, ps2_of, m_outer=True)

    orig_to_json = nc.to_json_bytes
    nc.to_json_bytes = lambda: _split_multiwait_json(orig_to_json())

    _CACHE["nc"] = nc
    return nc


# ---------------------------------------------------------------- entry
def kernel(insample_y, base_w1, spline_w1, scaler1, base_w2, spline_w2,
           scaler2):
    from concourse.bass_utils import run_bass_kernel_spmd

    nc = _build_program()
    w1 = _wcat_swizzled(np.asarray(base_w1), np.asarray(spline_w1),
                        np.asarray(scaler1))
    w2 = _wcat_swizzled(np.asarray(base_w2), np.asarray(spline_w2),
                        np.asarray(scaler2))
    xs = _x_swizzled(np.asarray(insample_y))
    in_maps = [{"xT": xs[c], "w1": w1, "w2": w2} for c in range(NCORES)]
    res = run_bass_kernel_spmd(nc, in_maps, list(range(NCORES)))
    y = np.concatenate([res.results[c]["yT"] for c in range(NCORES)], axis=1)
    return np.ascontiguousarray(y.T, dtype=np.float32)


# revision 37
# speedup vs baseline: 1.0108x; 1.0108x over previous
"""KAN expert (2x KANLinear, grid=5, order=3) on 8 TRN2 NeuronCores.

Algorithm
---------
KANLinear(x) = silu(x) @ base_w.T + einsum('big,oig->bo', B(x), spline_w*scaler)
where B(x) are 8 cubic B-spline bases on the uniform knot grid
t_j = -2.2 + 0.4*j, j=0..11.

Spline part reformulated with truncated powers: for x clamped to
(-inf, 2.2], B_g(x) = (1/(6 h^3)) * sum_{k=0..4} (-1)^k C(4,k) q_{g+k}(x)
with q_j(x) = relu(min(x, 2.2) - t_j)^3 and q_11 == 0. The 5-term
combination is folded into the spline weights on the host, so the device
computes, per feature, 12 channels (silu + q_0..q_10) and one matmul per
layer: out = concat_ch[silu(x), q_0..q_10] @ Wcat  (K = 12*in).

Sharding: data-parallel over batch (16384 -> 2048 rows/core), weights
replicated. Matmul operands fp16 (full PE rate), PSUM fp32.

Per-core pipeline over batch-column segments (256/512 wide), activations
transposed [feature, batch]; elementwise ops fused over feature blocks
([128, FB, seg] tiles). Per q channel: r = relu(xc-t) (DVE tensor_scalar,
4x mode), square s2 = (xc-t)^2 on ACT (Square with bias) or GPSIMD
(tt r*r), final q = s2*r on DVE or GPSIMD per a static balance table
that keeps every engine below the PE's ~31 us/chunk. Matmul emission is
software-pipelined: mm_l1(n+1) issues before mm_l2(n) so the PE never
waits for layer-2 channels of segment n (they are computed by
DVE/ACT/GPSIMD while mm_l1(n+1) streams).

This walrus build only supports ONE sync-wait per instruction; Tile
emits more. `_split_multiwait_json` rewrites the serialized BIR to hoist
excess waits onto injected NoOps.
"""

import json

import numpy as np

# ---------------------------------------------------------------- constants
B, IN, HID, OUT = 16384, 512, 256, 256
NCORES = 8
BPC = B // NCORES            # 2048 batch rows per core
# batch-column segments per core: narrow at the ends so the PE pipeline
# ramps in/out quickly, wide in the middle for low per-op overhead.
SEGS = [256, 512, 512, 512, 256]
assert sum(SEGS) == BPC
GRID_H = 0.4
T0 = -2.2
CLAMP_HI = 2.2
NCH = 11                     # truncated-power channels j = 0..10
KT1 = (1 + NCH) * IN // 128  # 48 k-tiles, layer 1
KT2 = (1 + NCH) * HID // 128  # 24 k-tiles, layer 2
FB1 = IN // 128              # 4 feature blocks, layer 1
FB2 = HID // 128             # 2 feature blocks, layer 2

_KNOTS = [T0 + GRID_H * j for j in range(12)]

# Per-channel engine assignment: (square_engine, mult_engine) per layer.
# square: 'A' = ACT Square(xc + bias); 'P' = GPSIMD TT(r, r).
# mult:   'D' = DVE tensor_tensor;     'P' = GPSIMD tensor_tensor.
# r = relu(xc - t_j) is always a DVE tensor_scalar.
RECIPES = {
    1: [("P", "D"), ("A", "P"), ("P", "D"), ("A", "D"), ("P", "D"),
        ("A", "P"), ("P", "D"), ("A", "D"), ("P", "D"), ("A", "P"),
        ("A", "D")],
    2: [("P", "D"), ("A", "D"), ("P", "D"), ("A", "P"), ("P", "D"),
        ("A", "D"), ("P", "D"), ("A", "P"), ("A", "D"), ("P", "D"),
        ("A", "D")],
}

_CACHE = {}


# ---------------------------------------------------------------- host math
def _fold_weights(spline_w, scaler):
    """[o,i,8] spline weights * scaler -> [o,i,11] truncated-power weights."""
    sw = spline_w.astype(np.float64) * scaler.astype(np.float64)[..., None]
    comb = np.array([1.0, -4.0, 6.0, -4.0, 1.0], np.float64) / (6.0 * GRID_H**3)
    o, i, _ = sw.shape
    wf = np.zeros((o, i, 12), np.float64)
    for g in range(8):
        for k in range(5):
            wf[:, :, g + k] += comb[k] * sw[:, :, g]
    return wf[:, :, :NCH]  # q_11 == 0 after the clamp


def _wcat_swizzled(base_w, spline_w, scaler):
    """Concat [base; folded-spline] weights, channel-major rows, swizzled to
    the SBUF layout [128, KT*out] (per-partition contiguous DMA)."""
    o, i = base_w.shape
    rows = np.empty((1 + NCH, i, o), np.float64)
    rows[0] = base_w.astype(np.float64).T
    rows[1:] = _fold_weights(spline_w, scaler).transpose(2, 1, 0)
    wcat = rows.reshape((1 + NCH) * i, o)               # [K, o]
    kt = wcat.shape[0] // 128
    return np.ascontiguousarray(
        wcat.reshape(kt, 128, o).transpose(1, 0, 2).reshape(128, kt * o)
    ).astype(np.float16)


def _x_swizzled(x):
    """x [B, IN] fp32 -> per-core [128, FB1*BPC] fp16 transposed layout."""
    xt = np.ascontiguousarray(x.T.astype(np.float16))   # [IN, B]
    outs = []
    for c in range(NCORES):
        sh = xt[:, c * BPC:(c + 1) * BPC]               # [512, 2048]
        outs.append(np.ascontiguousarray(
            sh.reshape(FB1, 128, BPC).transpose(1, 0, 2).reshape(128, FB1 * BPC)))
    return outs


# ------------------------------------------------- BIR wait-limit workaround
def _split_multiwait_json(raw: bytes) -> bytes:
    """Walrus in this container encodes at most one sync-wait per ISA
    instruction. Hoist extra waits onto injected same-engine NoOps."""
    d = json.loads(raw)
    k = 0
    for fn in d["functions"]:
        for bb in fn["blocks"]:
            out = []
            for ins in bb["instructions"]:
                si = ins.get("sync_info")
                waits = (si or {}).get("on_wait") or []
                if len(waits) > 1:
                    for w in waits[:-1]:
                        k += 1
                        out.append({"debug": ins.get("debug", 0),
                                    "engine": ins["engine"], "ins": [],
                                    "outs": [], "name": f"antws-{k}",
                                    "opcode": "NoOp",
                                    "sync_info": {"on_update": [],
                                                  "on_wait": [w]}})
                    si["on_wait"] = [waits[-1]]
                out.append(ins)
            bb["instructions"] = out
    return json.dumps(d).encode()


# ---------------------------------------------------------------- program
def _build_program():
    if "nc" in _CACHE:
        return _CACHE["nc"]
    import concourse.bass as bass
    import concourse.mybir as mybir
    import concourse.tile as tile

    f16 = mybir.dt.float16
    f32 = mybir.dt.float32
    AF = mybir.ActivationFunctionType
    ALU = mybir.AluOpType

    nc = bass.Bass(target_bir_lowering=False)

    # const APs for activation float biases (walrus wants APs, not imms)
    def reg_const(v):
        v = float(v)
        if (f32, v) in nc.const_aps.aps:
            return
        t = nc.alloc_sbuf_tensor(f"constk_{len(nc.const_aps.aps)}", [128, 1], f32)
        nc.gpsimd.memset(t.ap(), v)
        nc.const_aps.aps[(f32, v)] = t.ap()

    for j in range(NCH):
        reg_const(-_KNOTS[j])
    nc.all_engine_barrier()

    x_d = nc.declare_dram_parameter("xT", [128, FB1 * BPC], f16, isOutput=False)
    w1_d = nc.declare_dram_parameter("w1", [128, KT1 * OUT], f16, isOutput=False)
    w2_d = nc.declare_dram_parameter("w2", [128, KT2 * OUT], f16, isOutput=False)
    y_d = nc.declare_dram_parameter("yT", [OUT, BPC], f32, isOutput=True)

    with tile.TileContext(nc) as tc:
        with (
            tc.tile_pool(name="wp", bufs=1) as wp,
            tc.tile_pool(name="xp", bufs=1) as xp,
            tc.tile_pool(name="ch1", bufs=18) as ch1p,   # [128,FB1,512] tiles
            tc.tile_pool(name="ch2", bufs=16) as ch2p,   # [128,FB2,512] tiles
            tc.tile_pool(name="aux", bufs=4) as auxp,
            tc.tile_pool(name="outp", bufs=1) as outp,
            tc.tile_pool(name="ps", bufs=2, space="PSUM") as psp,
        ):
            w1_sb = wp.tile([128, KT1, OUT], f16, tag="w1")
            w2_sb = wp.tile([128, KT2, OUT], f16, tag="w2")
            x_sb = xp.tile([128, FB1, BPC], f16, tag="x")

            # One sync DMA queue: x segment 0 first (unblocks silu/q of the
            # first segment), then w1 in k-ordered eighths interleaved with
            # the remaining x segments so early k-tile weights land first.
            x_src = x_d.rearrange("p (f b) -> p f b", b=BPC)
            seg_lo = [sum(SEGS[:i]) for i in range(len(SEGS))]

            def dma_x(n, eng=None):
                eng = eng or nc.sync
                lo, w = seg_lo[n], SEGS[n]
                if n == 0:
                    # fb halves so segment 0's silu can start after 1 KiB
                    for h in range(2):
                        eng.dma_start(
                            x_sb[:, 2 * h:2 * h + 2, lo:lo + w],
                            x_src[:, 2 * h:2 * h + 2, lo:lo + w])
                else:
                    eng.dma_start(x_sb[:, :, lo:lo + w],
                                  x_src[:, :, lo:lo + w])

            def dma_w1(kl, kh):
                nc.sync.dma_start(w1_sb[:, kl:kh, :],
                                  w1_d[:, kl * OUT:kh * OUT].rearrange(
                                      "p (k m) -> p k m", m=OUT))

            # One sync queue, ordered by first-use time: x seg 0, early w1
            # k-tiles, remaining x segments interleaved with the rest of
            # the weights.
            dma_x(0)
            dma_w1(0, 6)
            dma_w1(6, 12)
            dma_x(1)
            dma_w1(12, 18)
            dma_w1(18, 24)
            dma_x(2)
            for s in range(4, 8):
                dma_w1(6 * s, 6 * (s + 1))
            for s in range(2):
                kl, kh = s * (KT2 // 2), (s + 1) * (KT2 // 2)
                nc.sync.dma_start(w2_sb[:, kl:kh, :],
                                  w2_d[:, kl * OUT:kh * OUT].rearrange(
                                      "p (k m) -> p k m", m=OUT))
            for n in range(3, len(SEGS)):
                dma_x(n)

            def make_channels(layer, xb, chp, nfb, width, lbl, ramp=False):
                """xb: [128, nfb, width] fp16 source tile AP.
                Returns list of 12 [128, nfb, width] channel tiles.
                ramp: emit silu/clamp in fb halves so the first matmuls
                unblock as soon as the first half of x lands."""
                tiles = []
                s = chp.tile([128, nfb, width], f16, tag="ch",
                             name=f"silu_{lbl}")
                xc = auxp.tile([128, nfb, width], f16, tag=f"xc{layer}",
                               name=f"xc_{lbl}", bufs=2)
                if ramp and nfb % 2 == 0:
                    h = nfb // 2
                    for i in range(2):
                        nc.scalar.activation(s[:, i * h:(i + 1) * h, :],
                                             xb[:, i * h:(i + 1) * h, :],
                                             AF.Silu)
                        nc.vector.tensor_scalar_min(
                            xc[:, i * h:(i + 1) * h, :],
                            xb[:, i * h:(i + 1) * h, :], CLAMP_HI)
                else:
                    nc.scalar.activation(s[:], xb, AF.Silu)
                    nc.vector.tensor_scalar_min(xc[:], xb, CLAMP_HI)
                tiles.append(s)
                for j in range(NCH):
                    sq_eng, mul_eng = RECIPES[layer][j]
                    q = chp.tile([128, nfb, width], f16, tag="ch",
                                 name=f"q_{lbl}_{j}")
                    r = auxp.tile([128, nfb, width], f16, tag=f"r{layer}",
                                  name=f"r_{lbl}_{j}", bufs=2)
                    nc.vector.tensor_scalar(r[:], xc[:], float(_KNOTS[j]),
                                            0.0, ALU.subtract, ALU.max)
                    s2 = auxp.tile([128, nfb, width], f16, tag=f"s2{layer}",
                                   name=f"s2_{lbl}_{j}", bufs=2)
                    if sq_eng == "A":
                        nc.scalar.activation(s2[:], xc[:], AF.Square,
                                             bias=float(-_KNOTS[j]))
                    else:
                        nc.gpsimd.tensor_tensor(s2[:], r[:], r[:], ALU.mult)
                    meng = nc.vector if mul_eng == "D" else nc.gpsimd
                    meng.tensor_tensor(q[:], s2[:], r[:], ALU.mult)
                    tiles.append(q)
                return tiles

            def mm(ps, w_sb, tiles, kt_count, nfb):
                for kt in range(kt_count):
                    ch, f = kt // nfb, kt % nfb
                    rhs = tiles[ch][:, f, :]
                    for m in range(2):
                        nc.tensor.matmul(ps[m][:],
                                         w_sb[:, kt, m * 128:(m + 1) * 128],
                                         rhs, start=(kt == 0),
                                         stop=(kt == kt_count - 1))

            def emit_l2_tail(n, ps2_of, m_outer=False):
                """mm_l2 for segment n, then y evacuation + store.
                m_outer: complete M-block 0's accumulation first so its
                evacuation overlaps M-block 1 (used for the last segment)."""
                ps2 = ps2_of[n]
                lo, w = seg_lo[n], SEGS[n]

                def evac(m):
                    yt = outp.tile([128, w], f32, tag=f"yt_{m}",
                                   name=f"yt_{n}_{m}")
                    nc.scalar.copy(yt[:], ps2[m][:])
                    nc.sync.dma_start(y_d[m * 128:(m + 1) * 128, lo:lo + w],
                                      yt[:])

                if m_outer:
                    for m in range(2):
                        for kt in range(KT2):
                            ch, f = kt // FB2, kt % FB2
                            nc.tensor.matmul(
                                ps2[m][:], w2_sb[:, kt, m * 128:(m + 1) * 128],
                                t2_of[n][ch][:, f, :], start=(kt == 0),
                                stop=(kt == KT2 - 1))
                        evac(m)
                else:
                    mm(ps2, w2_sb, t2_of[n], KT2, FB2)
                    for m in range(2):
                        evac(m)

            t2_of = {}
            ps2_of = {}
            for n, w in enumerate(SEGS):
                lo = seg_lo[n]
                # ---- layer-1 channels + matmul for segment n
                t1 = make_channels(1, x_sb[:, :, lo:lo + w], ch1p,
                                   FB1, w, f"l1c{n}", ramp=(n == 0))
                ps1 = [psp.tile([128, w], f32, tag=f"ps1_{m}",
                                name=f"ps1_{n}_{m}") for m in range(2)]
                mm(ps1, w1_sb, t1, KT1, FB1)
                # ---- layer-2 matmul of the PREVIOUS segment (PE
                # pipelining): its channels were computed while mm_l1(n)
                # streamed.
                if n >= 1:
                    emit_l2_tail(n - 1, ps2_of)
                # ---- evacuate ps1 -> x2, then layer-2 channels for seg n
                x2 = auxp.tile([128, FB2, w], f16, tag="x2",
                               name=f"x2_{n}", bufs=2)
                for m in range(2):
                    nc.scalar.copy(x2[:, m, :], ps1[m][:])
                t2_of[n] = make_channels(2, x2[:], ch2p, FB2, w, f"l2c{n}")
                ps2_of[n] = [psp.tile([128, w], f32, tag=f"ps2_{m}",
                                      name=f"ps2_{n}_{m}") for m in range(2)]
            emit_l2_tail(len(SEGS) - 1, ps2_of, m_outer=True)

    orig_to_json = nc.to_json_bytes
    nc.to_json_bytes = lambda: _split_multiwait_json(orig_to_json())

    _CACHE["nc"] = nc
    return nc


# ---------------------------------------------------------------- entry
def kernel(insample_y, base_w1, spline_w1, scaler1, base_w2, spline_w2,
           scaler2):
    from concourse.bass_utils import run_bass_kernel_spmd

    nc = _build_program()
    w1 = _wcat_swizzled(np.asarray(base_w1), np.asarray(spline_w1),
                        np.asarray(scaler1))
    w2 = _wcat_swizzled(np.asarray(base_w2), np.asarray(spline_w2),
                        np.asarray(scaler2))
    xs = _x_swizzled(np.asarray(insample_y))
    in_maps = [{"xT": xs[c], "w1": w1, "w2": w2} for c in range(NCORES)]
    res = run_bass_kernel_spmd(nc, in_maps, list(range(NCORES)))
    y = np.concatenate([res.results[c]["yT"] for c in range(NCORES)], axis=1)
    return np.ascontiguousarray(y.T, dtype=np.float32)
